# revision 22
# baseline (speedup 1.0000x reference)
"""Trainium2 Bass kernel for nn_EquivariantProductBasisWithSelfMagmomBlock, v2.

Feature-major design: host pre-transposes node tensors to [feat, node] layout
so the device does zero transposes. Per core: 8192 nodes in 16 supertiles of
512. All matmuls are f32r with 512-wide moving operands (full PE rate).
Sums are accumulated on the PE via identity-matmuls into PSUM; elementwise
muls are split across DVE (broadcast + PSUM-reading ops) and Pool (SBUF-only
ops); ACT does Square/Silu and all PSUM->SBUF copies.
"""

import sys

sys.path.insert(0, "/opt/trn_rl_repo")

from contextlib import ExitStack

import numpy as np

import concourse.bass as bass
import concourse.tile as tile
from concourse import bacc, mybir
from concourse.bass_utils import run_bass_kernel_spmd
from concourse.masks import make_identity

FP32 = mybir.dt.float32
F32R = mybir.dt.float32r
AF = mybir.ActivationFunctionType
OP = mybir.AluOpType

N = 65536
C = 128
E = 10
INV = 16
N_CORES = 8
N_CORE = N // N_CORES  # 8192
P = 128
ST = 512  # nodes per supertile


def r(ap):
    """bitcast an AP to float32r for full-rate fp32 matmul."""
    return ap.bitcast(F32R)


def bcast_mid(ap, k):
    """[P, W] AP -> [P, k, W] AP broadcasting over a new middle dim."""
    return bass.AP(tensor=ap.tensor, offset=ap.offset, ap=[ap.ap[0], [0, k]] + ap.ap[1:])


def build_program(n_st):
    nc = bacc.Bacc(
        "TRN2", target_bir_lowering=False, debug=False, num_devices=N_CORES
    )
    n_nodes = n_st * ST

    def din(name, shape):
        return nc.dram_tensor(name, list(shape), FP32, kind="ExternalInput").ap()

    nfT_d = din("nfT", (4, C, n_nodes))
    scT_d = din("scT", (4, C, n_nodes))
    smallT_d = din("smallT", (68, n_nodes))  # attrsT @0 | invT @32 | magT @64
    wsc_d = din("wsc", (E, 1152))  # w_sc0 [10,640] | w_sc1 [10,512], path-major
    w1_d = din("w1", (INV, 64))
    w234_d = din("w234", (64, 640))  # w2 | w3 | w4
    wf_d = din("wf", (C, 768))  # WA0 WB0 WO0 WA1 WB1 WO1
    outT_d = nc.dram_tensor("outT", [4, C, n_nodes], FP32, kind="ExternalOutput").ap()

    nf_r = nfT_d.rearrange("j c n -> c j n")
    sc_r = scT_d.rearrange("j c n -> c j n")
    out_r = outT_d.rearrange("j c n -> c j n")

    with tile.TileContext(nc) as tc, ExitStack() as ctx:
        singles = ctx.enter_context(tc.tile_pool(name="singles", bufs=1))
        # streaming SBUF pools (double-buffered across supertiles)
        stream = ctx.enter_context(tc.tile_pool(name="stream", bufs=2))
        # single-buffered intra-supertile temporaries
        tmp = ctx.enter_context(tc.tile_pool(name="tmp", bufs=1))
        # PSUM pools
        accA = ctx.enter_context(tc.tile_pool(name="accA", bufs=3, space="PSUM"))
        op_ps = ctx.enter_context(tc.tile_pool(name="op_ps", bufs=2, space="PSUM"))
        wzp = ctx.enter_context(tc.tile_pool(name="wzp", bufs=2, space="PSUM"))
        mlpp = ctx.enter_context(tc.tile_pool(name="mlpp", bufs=1, space="PSUM"))

        # ---------------- preloads ----------------
        ident_f = singles.tile([P, P], FP32)
        make_identity(nc, ident_f[:])
        ident = singles.tile([P, P], F32R)
        nc.vector.tensor_copy(ident[:], ident_f[:])

        # em[k, u*128+v] = 1 iff u == k, stored at partition base 64 to match magT
        em_padf = singles.tile([68, ST], FP32)
        em_f = em_padf[64:68, :]
        nc.gpsimd.memset(em_f, 0.0)
        nc.gpsimd.affine_select(
            out=em_f,
            in_=em_f,
            compare_op=OP.not_equal,
            fill=1.0,
            base=0,
            pattern=[[1, 4], [0, P]],
            channel_multiplier=-1,
        )
        em_pad = singles.tile([68, ST], F32R)
        nc.vector.tensor_copy(em_pad[64:68, :], em_f)
        em = em_pad[64:68, :]

        stage = singles.tile([P, 1152], FP32)
        wsc_sb = singles.tile([E, 1152], F32R)
        nc.sync.dma_start(out=stage[0:E, 0:1152], in_=wsc_d)
        nc.vector.tensor_copy(wsc_sb[:], stage[0:E, 0:1152])
        w1_pad = singles.tile([32 + INV, 64], F32R)
        nc.sync.dma_start(out=stage[32 : 32 + INV, 0:64], in_=w1_d)
        nc.vector.tensor_copy(w1_pad[32 : 32 + INV, :], stage[32 : 32 + INV, 0:64])
        w1_sb = w1_pad[32 : 32 + INV, :]
        w234_sb = singles.tile([64, 640], F32R)
        nc.sync.dma_start(out=stage[0:64, 128:768], in_=w234_d)
        nc.vector.tensor_copy(w234_sb[:], stage[0:64, 128:768])
        wf_sb = singles.tile([C, 768], F32R)
        nc.sync.dma_start(out=stage[:, 384:1152], in_=wf_d)
        nc.vector.tensor_copy(wf_sb[:], stage[:, 384:1152])

        WA0 = wf_sb[:, 0:128]
        WB0 = wf_sb[:, 128:256]
        WO0 = wf_sb[:, 256:384]
        WA1 = wf_sb[:, 384:512]
        WB1 = wf_sb[:, 512:640]
        WO1 = wf_sb[:, 640:768]

        pending_out = None
        for s_ in range(n_st):
            lo = s_ * ST
            # ---------------- loads ----------------
            x_sb = stream.tile([P, 4, ST], FP32, tag="x")
            nc.sync.dma_start(out=x_sb[:], in_=nf_r[:, :, lo : lo + ST])
            sc_sb = stream.tile([P, 4, ST], FP32, tag="sc")
            nc.sync.dma_start(out=sc_sb[:], in_=sc_r[:, :, lo : lo + ST])
            small_sb = stream.tile([68, ST], FP32, tag="small")
            nc.sync.dma_start(out=small_sb[:], in_=smallT_d[:, lo : lo + ST])
            # issue the PREVIOUS supertile's store now so its sem wait never
            # blocks this supertile's loads on the SP queue
            if pending_out is not None:
                po_sb, po_lo = pending_out
                nc.sync.dma_start(out=out_r[:, :, po_lo : po_lo + ST], in_=po_sb[:])
            small_r = tmp.tile([68, ST], F32R, tag="small_r")
            nc.scalar.copy(small_r[:], small_sb[:])
            attrs_T = small_r[0:E, :]
            inv_T = small_r[32 : 32 + INV, :]
            magT = small_r[64:68, :]

            x0 = x_sb[:, 0, :]
            x1 = x_sb[:, 1:4, :]

            # ---------------- MLP first; replicates + a-copies interleaved --
            a_sb = stream.tile([P, 4, ST], FP32, tag="a")
            reps = []
            for m in range(4):
                rp = accA.tile([P, ST], FP32, tag="accA")
                nc.tensor.matmul(rp[:], em[:, m * P : (m + 1) * P], magT)
                reps.append(rp)
            a0 = a_sb[:, 0, :]
            a1 = a_sb[:, 1:4, :]

            h1p = mlpp.tile([64, ST], FP32, tag="h")
            nc.tensor.matmul(h1p[:], w1_sb, inv_T)
            h1 = tmp.tile([64, ST], F32R, tag="h1")
            nc.scalar.activation(h1[:], h1p[:], AF.Silu)
            nc.scalar.copy(a_sb[:, 0, :], reps[0][:])
            nc.scalar.copy(a_sb[:, 1, :], reps[1][:])
            h2p = mlpp.tile([64, ST], FP32, tag="h")
            nc.tensor.matmul(h2p[:], w234_sb[:, 0:64], h1[:])
            h2 = tmp.tile([64, ST], F32R, tag="h2")
            nc.scalar.activation(h2[:], h2p[:], AF.Silu)
            nc.scalar.copy(a_sb[:, 2, :], reps[2][:])
            nc.scalar.copy(a_sb[:, 3, :], reps[3][:])
            h3p = mlpp.tile([64, ST], FP32, tag="h")
            nc.tensor.matmul(h3p[:], w234_sb[:, 64:128], h2[:])
            h3 = tmp.tile([64, ST], F32R, tag="h3")
            nc.scalar.activation(h3[:], h3p[:], AF.Silu)

            # ---------------- monomials ----------------
            sq_sb = tmp.tile([P, 2, ST], FP32, tag="sq")
            nc.scalar.activation(sq_sb[:, 0, :], x0, AF.Square)
            nc.gpsimd.tensor_mul(sq_sb[:, 1, :], x_sb[:, 1, :], x_sb[:, 1, :])
            x0sq = sq_sb[:, 0, :]

            n1a = tmp.tile([P, 2, ST], FP32, tag="n1a")
            nc.gpsimd.tensor_mul(n1a[:, 0, :], x_sb[:, 2, :], x_sb[:, 2, :])
            nc.gpsimd.tensor_mul(n1a[:, 1, :], x_sb[:, 3, :], x_sb[:, 3, :])
            n1b = tmp.tile([P, ST], FP32, tag="n1b")
            nc.gpsimd.tensor_add(n1b[:], sq_sb[:, 1, :], n1a[:, 0, :])
            n1 = tmp.tile([P, ST], FP32, tag="n1", bufs=2)
            nc.gpsimd.tensor_add(n1[:], n1b[:], n1a[:, 1, :])

            # xa = sum_m a1m * x1m (independent of c1, so the late
            # s = c1*xa becomes a single op instead of a 5-op tail)
            xam = tmp.tile([P, 3, ST], FP32, tag="sm")
            nc.gpsimd.tensor_mul(xam[:, 0, :], x_sb[:, 1, :], a_sb[:, 1, :])
            nc.gpsimd.tensor_mul(xam[:, 1, :], x_sb[:, 2, :], a_sb[:, 2, :])
            nc.gpsimd.tensor_mul(xam[:, 2, :], x_sb[:, 3, :], a_sb[:, 3, :])
            xa2 = tmp.tile([P, ST], FP32, tag="sa")
            nc.gpsimd.tensor_add(xa2[:], xam[:, 0, :], xam[:, 1, :])
            xa = tmp.tile([P, ST], FP32, tag="xa", bufs=2)
            nc.gpsimd.tensor_add(xa[:], xa2[:], xam[:, 2, :])

            # ---------------- y0 via Horner: y0 = x0*A + n1*B ----------------
            # A = wz0_0 + wz0_1*x0 + wz0_3*x0sq ; B = wz0_2 + wz0_4*x0
            Ap = accA.tile([P, ST], FP32, tag="accA")
            nc.tensor.matmul(Ap[:], wsc_sb[:, 0:128], attrs_T, start=True, stop=False)
            Bp = accA.tile([P, ST], FP32, tag="accA")
            nc.tensor.matmul(Bp[:], wsc_sb[:, 256:384], attrs_T, start=True, stop=False)
            wz1t = wzp.tile([P, ST], FP32, tag="wz")
            nc.tensor.matmul(wz1t[:], wsc_sb[:, 128:256], attrs_T)
            u1 = tmp.tile([P, ST], F32R, tag="u1", bufs=2)
            nc.vector.tensor_mul(u1[:], wz1t[:], x0)
            wz3t = wzp.tile([P, ST], FP32, tag="wz")
            nc.tensor.matmul(wz3t[:], wsc_sb[:, 384:512], attrs_T)
            u2 = tmp.tile([P, ST], F32R, tag="u2", bufs=2)
            nc.vector.tensor_mul(u2[:], wz3t[:], x0sq)
            wz4t = wzp.tile([P, ST], FP32, tag="wz")
            nc.tensor.matmul(wz4t[:], wsc_sb[:, 512:640], attrs_T)
            u3 = tmp.tile([P, ST], F32R, tag="u3")
            nc.vector.tensor_mul(u3[:], wz4t[:], x0)
            nc.tensor.matmul(Ap[:], ident[:], u1[:], start=False, stop=False)
            nc.tensor.matmul(Ap[:], ident[:], u2[:], start=False, stop=True)
            nc.tensor.matmul(Bp[:], ident[:], u3[:], start=False, stop=True)
            xA = tmp.tile([P, ST], F32R, tag="xA")
            nc.vector.tensor_mul(xA[:], Ap[:], x0)
            nB = tmp.tile([P, ST], F32R, tag="nB")
            nc.vector.tensor_mul(nB[:], Bp[:], n1[:])
            y0p = accA.tile([P, ST], FP32, tag="accA")
            nc.tensor.matmul(y0p[:], ident[:], xA[:], start=True, stop=False)
            nc.tensor.matmul(y0p[:], ident[:], nB[:], start=False, stop=True)
            y0 = tmp.tile([P, ST], F32R, tag="y0", bufs=2)
            nc.scalar.copy(y0[:], y0p[:])

            # ---------------- c1 = wz1_0 + wz1_1 x0 + wz1_2 x0^2 + wz1_3 n1 -
            c1p = accA.tile([P, ST], FP32, tag="accA")
            nc.tensor.matmul(
                c1p[:], wsc_sb[:, 640:768], attrs_T, start=True, stop=False
            )
            cmon = [x0, x0sq, n1[:]]
            ms = []
            for i_ in range(3):
                wz = wzp.tile([P, ST], FP32, tag="wz")
                nc.tensor.matmul(
                    wz[:], wsc_sb[:, 768 + i_ * P : 768 + (i_ + 1) * P], attrs_T
                )
                mt = tmp.tile([P, ST], F32R, tag=f"m{i_}", bufs=2)
                nc.vector.tensor_mul(mt[:], wz[:], cmon[i_])
                ms.append(mt)
            for i_ in range(3):
                nc.tensor.matmul(
                    c1p[:], ident[:], ms[i_][:],
                    start=False, stop=(i_ == 2),
                )
            c1 = tmp.tile([P, ST], FP32, tag="c1")
            nc.scalar.copy(c1[:], c1p[:])

            # ---------------- y1m = c1 * x1m ----------------
            y1 = tmp.tile([P, 3, ST], F32R, tag="y1", bufs=2)
            nc.vector.tensor_mul(y1[:], bcast_mid(c1p[:], 3), x1)

            # ---------------- s = c1 * xa ----------------
            s_ = tmp.tile([P, ST], FP32, tag="s_")
            nc.vector.tensor_mul(s_[:], c1p[:], xa[:])

            # tpw blocks wa/wb/wc/wd stay in PSUM; DVE consumers read them there
            tpw_ps = []
            for k in range(4):
                tp = wzp.tile([P, ST], FP32, tag="wz")
                nc.tensor.matmul(
                    tp[:], w234_sb[:, 128 + k * P : 128 + (k + 1) * P], h3[:]
                )
                tpw_ps.append(tp)
            wap, wbp, wcp, wdp = tpw_ps

            # ---------------- TP products ----------------
            wc = tmp.tile([P, ST], FP32, tag="wc_sb", bufs=2)
            nc.scalar.copy(wc[:], wcp[:])
            ay0 = tmp.tile([P, ST], FP32, tag="ay0")
            nc.gpsimd.tensor_mul(ay0[:], a0, y0[:])
            g1a = tmp.tile([P, ST], F32R, tag="g1a", bufs=2)
            nc.vector.tensor_mul(g1a[:], ay0[:], wap[:])
            g2 = tmp.tile([P, ST], F32R, tag="g2", bufs=2)
            nc.vector.tensor_mul(g2[:], wbp[:], s_[:])
            pc2 = tmp.tile([P, ST], FP32, tag="pc2")
            nc.gpsimd.tensor_mul(pc2[:], wc[:], y0[:])
            m1c = tmp.tile([P, 3, ST], F32R, tag="m1c")
            for m in range(3):
                nc.gpsimd.tensor_mul(m1c[:, m, :], pc2[:], a_sb[:, 1 + m, :])
            rc = tmp.tile([P, ST], FP32, tag="rc")
            nc.vector.tensor_mul(rc[:], wdp[:], a0)
            rc2 = tmp.tile([P, ST], FP32, tag="rc2")
            nc.gpsimd.tensor_mul(rc2[:], rc[:], c1[:])
            hm = tmp.tile([P, 3, ST], F32R, tag="hm")
            for m in range(3):
                nc.gpsimd.tensor_mul(hm[:, m, :], rc2[:], x_sb[:, 1 + m, :])

            # ---------------- output linears (feature-major accumulation) ---
            outT_sb = stream.tile([P, 4, ST], FP32, tag="outT")
            o0 = op_ps.tile([P, ST], FP32, tag="op")
            nc.tensor.matmul(o0[:], WA0, g1a[:], start=True, stop=False)
            nc.tensor.matmul(o0[:], WB0, g2[:], start=False, stop=False)
            nc.tensor.matmul(o0[:], WO0, y0[:], start=False, stop=True)
            nc.vector.tensor_add(outT_sb[:, 0, :], o0[:], sc_sb[:, 0, :])
            for m in range(3):
                om = op_ps.tile([P, ST], FP32, tag="op")
                nc.tensor.matmul(om[:], WA1, m1c[:, m, :], start=True, stop=False)
                nc.tensor.matmul(om[:], WB1, hm[:, m, :], start=False, stop=False)
                nc.tensor.matmul(om[:], WO1, y1[:, m, :], start=False, stop=True)
                oc = tmp.tile([P, ST], FP32, tag=f"oc{m}")
                nc.scalar.copy(oc[:], om[:])
                nc.gpsimd.tensor_add(outT_sb[:, 1 + m, :], oc[:], sc_sb[:, 1 + m, :])

            pending_out = (outT_sb, lo)

        po_sb, po_lo = pending_out
        nc.sync.dma_start(out=out_r[:, :, po_lo : po_lo + ST], in_=po_sb[:])

    nc.compile()
    return nc


_CACHE = {}


def _get_program(n_st):
    if n_st not in _CACHE:
        _CACHE[n_st] = build_program(n_st)
    return _CACHE[n_st]


def _in_map_for_core(inputs, c, n_core):
    lo, hi = c * n_core, (c + 1) * n_core
    nf = np.asarray(inputs["node_feats"][lo:hi], dtype=np.float32)  # [n, C, 4]
    sc = np.asarray(inputs["sc"][lo:hi], dtype=np.float32)  # [n, 4C]
    nfT = np.ascontiguousarray(nf.transpose(2, 1, 0))  # [4, C, n]
    sc0 = sc[:, :C].T[None]  # [1, C, n]
    sc1 = np.ascontiguousarray(sc[:, C:].reshape(-1, C, 3).transpose(2, 1, 0))
    scT = np.ascontiguousarray(np.concatenate([sc0, sc1], axis=0))  # [4, C, n]
    n_loc = hi - lo
    smallT = np.zeros((68, n_loc), dtype=np.float32)
    smallT[0:E] = np.asarray(inputs["node_attrs"][lo:hi], dtype=np.float32).T
    smallT[32 : 32 + INV] = np.asarray(
        inputs["magmom_node_inv_feats"][lo:hi], dtype=np.float32
    ).T
    smallT[64:68] = np.asarray(inputs["magmom_node_attrs"][lo:hi], dtype=np.float32).T
    wsc = np.ascontiguousarray(
        np.concatenate(
            [
                np.asarray(inputs["w_sc0"], dtype=np.float32).reshape(E, 5 * C),
                np.asarray(inputs["w_sc1"], dtype=np.float32).reshape(E, 4 * C),
            ],
            axis=1,
        )
    )  # [10, 1152]
    w234 = np.ascontiguousarray(
        np.concatenate(
            [
                np.asarray(inputs["w_mlp2"], dtype=np.float32),
                np.asarray(inputs["w_mlp3"], dtype=np.float32),
                np.asarray(inputs["w_mlp4"], dtype=np.float32),
            ],
            axis=1,
        )
    )  # [64, 640]
    wl0 = np.asarray(inputs["W_l0"], dtype=np.float32)
    wl1 = np.asarray(inputs["W_l1"], dtype=np.float32)
    wf = np.ascontiguousarray(
        np.concatenate(
            [
                wl0[:C],
                wl0[C:],
                np.asarray(inputs["Wo0"], dtype=np.float32),
                wl1[:C],
                wl1[C:],
                np.asarray(inputs["Wo1"], dtype=np.float32),
            ],
            axis=1,
        )
    )  # [128, 768]
    return {
        "nfT": nfT,
        "scT": scT,
        "smallT": smallT,
        "wsc": wsc,
        "w1": np.ascontiguousarray(np.asarray(inputs["w_mlp1"], dtype=np.float32)),
        "w234": w234,
        "wf": wf,
    }


def _out_from_core(outT):
    """outT [4, C, n] -> out [n, 4C] in reference layout."""
    n = outT.shape[2]
    out = np.empty((n, 4 * C), dtype=np.float32)
    out[:, :C] = outT[0].T
    out[:, C:] = outT[1:4].transpose(2, 1, 0).reshape(n, 3 * C)
    return out


def run_on_hw(inputs, trace=False):
    n_nodes = np.asarray(inputs["node_feats"]).shape[0]
    n_core = n_nodes // N_CORES
    nc = _get_program(n_core // ST)
    in_maps = [_in_map_for_core(inputs, c, n_core) for c in range(N_CORES)]
    res = run_bass_kernel_spmd(
        nc, in_maps, core_ids=list(range(N_CORES)), trace=trace
    )
    out = np.concatenate(
        [_out_from_core(res.results[c]["outT"]) for c in range(N_CORES)], axis=0
    )
    return out.astype(np.float32), res


def kernel(**inputs) -> np.ndarray:
    import os, time

    os.environ.setdefault("NEURON_RT_RESET_CORES", "1")
    last = None
    for attempt in range(3):
        try:
            out, _ = run_on_hw(inputs, trace=False)
            return out
        except Exception as e:
            last = e
            time.sleep(5 + 5 * attempt)
    raise last


def bench(inputs, iters=5):
    """Pipelined timing of the sharded NEFF execution (device-resident inputs)."""
    import time
    import jax
    from jax.sharding import Mesh, PartitionSpec
    from jax.experimental.shard_map import shard_map
    from concourse import bass2jax
    from concourse.bass2jax import _bass_exec_p, install_neuronx_cc_hook

    inputs = {k: np.asarray(v, dtype=np.float32) for k, v in inputs.items()}
    n_nodes = inputs["node_feats"].shape[0]
    n_core = n_nodes // N_CORES
    nc = _get_program(n_core // ST)
    in_maps = [_in_map_for_core(inputs, c, n_core) for c in range(N_CORES)]

    install_neuronx_cc_hook()
    partition_name = nc.partition_id_tensor.name if nc.partition_id_tensor else None
    in_names, out_names, out_avals, zero_outs = [], [], [], []
    for alloc in nc.m.functions[0].allocations:
        if not isinstance(alloc, mybir.MemoryLocationSet):
            continue
        name = alloc.memorylocations[0].name
        if alloc.kind == "ExternalInput":
            if name != partition_name:
                in_names.append(name)
        elif alloc.kind == "ExternalOutput":
            out_names.append(name)
            shape = tuple(alloc.tensor_shape)
            dtype = mybir.dt.np(alloc.dtype)
            out_avals.append(jax.core.ShapedArray(shape, dtype))
            zero_outs.append(np.zeros(shape, dtype))
    n_params = len(in_names)
    all_names = in_names + out_names
    if partition_name is not None:
        all_names.append(partition_name)

    def _body(*args):
        operands = list(args)
        if partition_name is not None:
            operands.append(bass2jax.partition_id_tensor())
        return tuple(
            _bass_exec_p.bind(
                *operands,
                out_avals=tuple(out_avals),
                in_names=tuple(all_names),
                out_names=tuple(out_names),
                lowering_input_output_aliases=(),
                sim_require_finite=True,
                sim_require_nnan=True,
                nc=nc,
            )
        )

    devices = jax.devices()[:N_CORES]
    mesh = Mesh(np.asarray(devices), ("core",))
    nin = n_params + len(out_names)
    sharded = jax.jit(
        shard_map(
            _body,
            mesh=mesh,
            in_specs=(PartitionSpec("core"),) * nin,
            out_specs=(PartitionSpec("core"),) * len(out_names),
            check_rep=False,
        ),
        keep_unused=True,
    )
    per_core = [[np.asarray(m[nm]) for nm in in_names] for m in in_maps]
    concat_in = [
        np.concatenate([per_core[c][i] for c in range(N_CORES)], axis=0)
        for i in range(n_params)
    ]
    concat_zeros = [
        np.zeros((N_CORES * z.shape[0], *z.shape[1:]), z.dtype) for z in zero_outs
    ]
    from jax.sharding import NamedSharding
    sh = NamedSharding(mesh, PartitionSpec("core"))
    dev_in = [jax.device_put(a, sh) for a in concat_in + concat_zeros]
    out = sharded(*dev_in)
    jax.block_until_ready(out)
    t0 = time.time()
    for _ in range(iters):
        out = sharded(*dev_in)
    jax.block_until_ready(out)
    dt = (time.time() - t0) / iters
    return dt * 1e9, out



def bench_chain(inputs, n_chain=16, loops=3):
    """Per-execution time with N NEFF executions chained inside ONE jitted
    program: amortizes host/tunnel dispatch so total/N approaches true device
    execution time. Returns best ns/exec over `loops` timed dispatches."""
    import time
    import jax
    from jax.sharding import Mesh, PartitionSpec, NamedSharding
    from jax.experimental.shard_map import shard_map
    from concourse import bass2jax
    from concourse.bass2jax import _bass_exec_p, install_neuronx_cc_hook

    inputs = {k: np.asarray(v, dtype=np.float32) for k, v in inputs.items()}
    n_nodes = np.asarray(inputs["node_feats"]).shape[0]
    n_core = n_nodes // N_CORES
    nc = _get_program(n_core // ST)
    in_maps = [_in_map_for_core(inputs, c, n_core) for c in range(N_CORES)]

    install_neuronx_cc_hook()
    partition_name = nc.partition_id_tensor.name if nc.partition_id_tensor else None
    in_names, out_names, out_avals, zero_outs = [], [], [], []
    for alloc in nc.m.functions[0].allocations:
        if not isinstance(alloc, mybir.MemoryLocationSet):
            continue
        name = alloc.memorylocations[0].name
        if alloc.kind == "ExternalInput":
            if name != partition_name:
                in_names.append(name)
        elif alloc.kind == "ExternalOutput":
            out_names.append(name)
            shape = tuple(alloc.tensor_shape)
            dtype = mybir.dt.np(alloc.dtype)
            out_avals.append(jax.core.ShapedArray(shape, dtype))
            zero_outs.append(np.zeros(shape, dtype))
    n_params = len(in_names)
    all_names = in_names + out_names
    if partition_name is not None:
        all_names.append(partition_name)

    def _one(args):
        operands = list(args)
        if partition_name is not None:
            operands.append(bass2jax.partition_id_tensor())
        return _bass_exec_p.bind(
            *operands,
            out_avals=tuple(out_avals),
            in_names=tuple(all_names),
            out_names=tuple(out_names),
            lowering_input_output_aliases=(),
            sim_require_finite=True,
            sim_require_nnan=True,
            nc=nc,
        )

    def _body(*args):
        outs = None
        for _ in range(n_chain):
            outs = _one(args)
        return tuple(outs)

    devices = jax.devices()[:N_CORES]
    mesh = Mesh(np.asarray(devices), ("core",))
    nin = n_params + len(out_names)
    sharded = jax.jit(
        shard_map(
            _body,
            mesh=mesh,
            in_specs=(PartitionSpec("core"),) * nin,
            out_specs=(PartitionSpec("core"),) * len(out_names),
            check_rep=False,
        ),
        keep_unused=True,
    )
    per_core = [[np.asarray(m[nm]) for nm in in_names] for m in in_maps]
    concat_in = [
        np.concatenate([per_core[c][i] for c in range(N_CORES)], axis=0)
        for i in range(n_params)
    ]
    concat_zeros = [
        np.zeros((N_CORES * z.shape[0], *z.shape[1:]), z.dtype) for z in zero_outs
    ]
    sh = NamedSharding(mesh, PartitionSpec("core"))
    dev_in = [jax.device_put(a, sh) for a in concat_in + concat_zeros]
    out = sharded(*dev_in)
    jax.block_until_ready(out)
    best = None
    for _ in range(loops):
        t0 = time.time()
        out = sharded(*dev_in)
        jax.block_until_ready(out)
        dt = (time.time() - t0) / n_chain
        if best is None or dt < best:
            best = dt
    return best * 1e9, out


# revision 27
# speedup vs baseline: 1.5968x; 1.5968x over previous
"""Trainium2 Bass kernel for nn_EquivariantProductBasisWithSelfMagmomBlock, v2.

Feature-major design: host pre-transposes node tensors to [feat, node] layout
so the device does zero transposes. Per core: 8192 nodes in 16 supertiles of
512. All matmuls are f32r with 512-wide moving operands (full PE rate).
Sums are accumulated on the PE via identity-matmuls into PSUM; elementwise
muls are split across DVE (broadcast + PSUM-reading ops) and Pool (SBUF-only
ops); ACT does Square/Silu and all PSUM->SBUF copies.
"""

import sys

sys.path.insert(0, "/opt/trn_rl_repo")

from contextlib import ExitStack

import numpy as np

import concourse.bass as bass
import concourse.tile as tile
from concourse import bacc, mybir
from concourse.bass_utils import run_bass_kernel_spmd
from concourse.masks import make_identity

FP32 = mybir.dt.float32
F32R = mybir.dt.float32r
AF = mybir.ActivationFunctionType
OP = mybir.AluOpType

N = 65536
C = 128
E = 10
INV = 16
N_CORES = 8
N_CORE = N // N_CORES  # 8192
P = 128
ST = 512  # nodes per supertile


def r(ap):
    """bitcast an AP to float32r for full-rate fp32 matmul."""
    return ap.bitcast(F32R)


def bcast_mid(ap, k):
    """[P, W] AP -> [P, k, W] AP broadcasting over a new middle dim."""
    return bass.AP(tensor=ap.tensor, offset=ap.offset, ap=[ap.ap[0], [0, k]] + ap.ap[1:])


def build_program(n_st):
    nc = bacc.Bacc(
        "TRN2", target_bir_lowering=False, debug=False, num_devices=N_CORES
    )
    n_nodes = n_st * ST

    def din(name, shape):
        return nc.dram_tensor(name, list(shape), FP32, kind="ExternalInput").ap()

    nfT_d = din("nfT", (4, C, n_nodes))
    scT_d = din("scT", (4, C, n_nodes))
    smallT_d = din("smallT", (68, n_nodes))  # attrsT @0 | invT @32 | magT @64
    wsc_d = din("wsc", (E, 1152))  # w_sc0 [10,640] | w_sc1 [10,512], path-major
    w1_d = din("w1", (INV, 64))
    w234_d = din("w234", (64, 640))  # w2 | w3 | w4
    wf_d = din("wf", (C, 768))  # WA0 WB0 WO0 WA1 WB1 WO1
    outT_d = nc.dram_tensor("outT", [4, C, n_nodes], FP32, kind="ExternalOutput").ap()

    nf_r = nfT_d.rearrange("j c n -> c j n")
    sc_r = scT_d.rearrange("j c n -> c j n")
    out_r = outT_d.rearrange("j c n -> c j n")

    with tile.TileContext(nc) as tc, ExitStack() as ctx:
        singles = ctx.enter_context(tc.tile_pool(name="singles", bufs=1))
        # streaming SBUF pools (double-buffered across supertiles)
        stream = ctx.enter_context(tc.tile_pool(name="stream", bufs=2))
        # single-buffered intra-supertile temporaries
        tmp = ctx.enter_context(tc.tile_pool(name="tmp", bufs=1))
        # PSUM pools
        accA = ctx.enter_context(tc.tile_pool(name="accA", bufs=3, space="PSUM"))
        op_ps = ctx.enter_context(tc.tile_pool(name="op_ps", bufs=2, space="PSUM"))
        wzp = ctx.enter_context(tc.tile_pool(name="wzp", bufs=2, space="PSUM"))
        mlpp = ctx.enter_context(tc.tile_pool(name="mlpp", bufs=1, space="PSUM"))

        # ---------------- preloads ----------------
        ident_f = singles.tile([P, P], FP32)
        make_identity(nc, ident_f[:])
        ident = singles.tile([P, P], F32R)
        nc.vector.tensor_copy(ident[:], ident_f[:])

        # em[k, u*128+v] = 1 iff u == k, stored at partition base 64 to match magT
        em_padf = singles.tile([68, ST], FP32)
        em_f = em_padf[64:68, :]
        nc.gpsimd.memset(em_f, 0.0)
        nc.gpsimd.affine_select(
            out=em_f,
            in_=em_f,
            compare_op=OP.not_equal,
            fill=1.0,
            base=0,
            pattern=[[1, 4], [0, P]],
            channel_multiplier=-1,
        )
        em_pad = singles.tile([68, ST], F32R)
        nc.vector.tensor_copy(em_pad[64:68, :], em_f)
        em = em_pad[64:68, :]

        stage = singles.tile([P, 1152], FP32)
        wsc_sb = singles.tile([E, 1152], F32R)
        nc.sync.dma_start(out=stage[0:E, 0:1152], in_=wsc_d)
        nc.vector.tensor_copy(wsc_sb[:], stage[0:E, 0:1152])
        w1_pad = singles.tile([32 + INV, 64], F32R)
        nc.sync.dma_start(out=stage[32 : 32 + INV, 0:64], in_=w1_d)
        nc.vector.tensor_copy(w1_pad[32 : 32 + INV, :], stage[32 : 32 + INV, 0:64])
        w1_sb = w1_pad[32 : 32 + INV, :]
        w234_sb = singles.tile([64, 640], F32R)
        nc.sync.dma_start(out=stage[0:64, 128:768], in_=w234_d)
        nc.vector.tensor_copy(w234_sb[:], stage[0:64, 128:768])
        wf_sb = singles.tile([C, 768], F32R)
        nc.sync.dma_start(out=stage[:, 384:1152], in_=wf_d)
        nc.vector.tensor_copy(wf_sb[:], stage[:, 384:1152])

        WA0 = wf_sb[:, 0:128]
        WB0 = wf_sb[:, 128:256]
        WO0 = wf_sb[:, 256:384]
        WA1 = wf_sb[:, 384:512]
        WB1 = wf_sb[:, 512:640]
        WO1 = wf_sb[:, 640:768]

        pending_out = None
        for s_ in range(n_st):
            lo = s_ * ST
            # ---------------- loads ----------------
            x_sb = stream.tile([P, 4, ST], FP32, tag="x")
            nc.sync.dma_start(out=x_sb[:], in_=nf_r[:, :, lo : lo + ST])
            sc_sb = stream.tile([P, 4, ST], FP32, tag="sc")
            nc.sync.dma_start(out=sc_sb[:], in_=sc_r[:, :, lo : lo + ST])
            small_sb = stream.tile([68, ST], FP32, tag="small")
            nc.sync.dma_start(out=small_sb[:], in_=smallT_d[:, lo : lo + ST])
            # issue the PREVIOUS supertile's store now so its sem wait never
            # blocks this supertile's loads on the SP queue
            if pending_out is not None:
                po_sb, po_lo = pending_out
                nc.sync.dma_start(out=out_r[:, :, po_lo : po_lo + ST], in_=po_sb[:])
            small_r = tmp.tile([68, ST], F32R, tag="small_r")
            nc.scalar.copy(small_r[:], small_sb[:])
            attrs_T = small_r[0:E, :]
            inv_T = small_r[32 : 32 + INV, :]
            magT = small_r[64:68, :]

            x0 = x_sb[:, 0, :]
            x1 = x_sb[:, 1:4, :]

            # ---------------- MLP first; replicates + a-copies interleaved --
            a_sb = stream.tile([P, 4, ST], FP32, tag="a")
            reps = []
            for m in range(4):
                rp = accA.tile([P, ST], FP32, tag="accA")
                nc.tensor.matmul(rp[:], em[:, m * P : (m + 1) * P], magT)
                reps.append(rp)
            a0 = a_sb[:, 0, :]
            a1 = a_sb[:, 1:4, :]

            h1p = mlpp.tile([64, ST], FP32, tag="h")
            nc.tensor.matmul(h1p[:], w1_sb, inv_T)
            h1 = tmp.tile([64, ST], F32R, tag="h1")
            nc.scalar.activation(h1[:], h1p[:], AF.Silu)
            nc.scalar.copy(a_sb[:, 0, :], reps[0][:])
            nc.scalar.copy(a_sb[:, 1, :], reps[1][:])
            h2p = mlpp.tile([64, ST], FP32, tag="h")
            nc.tensor.matmul(h2p[:], w234_sb[:, 0:64], h1[:])
            h2 = tmp.tile([64, ST], F32R, tag="h2")
            nc.scalar.activation(h2[:], h2p[:], AF.Silu)
            nc.scalar.copy(a_sb[:, 2, :], reps[2][:])
            nc.scalar.copy(a_sb[:, 3, :], reps[3][:])
            h3p = mlpp.tile([64, ST], FP32, tag="h")
            nc.tensor.matmul(h3p[:], w234_sb[:, 64:128], h2[:])
            h3 = tmp.tile([64, ST], F32R, tag="h3")
            nc.scalar.activation(h3[:], h3p[:], AF.Silu)

            # ---------------- monomials ----------------
            sq_sb = tmp.tile([P, 2, ST], FP32, tag="sq")
            nc.scalar.activation(sq_sb[:, 0, :], x0, AF.Square)
            nc.gpsimd.tensor_mul(sq_sb[:, 1, :], x_sb[:, 1, :], x_sb[:, 1, :])
            x0sq = sq_sb[:, 0, :]

            n1a = tmp.tile([P, 2, ST], FP32, tag="n1a")
            nc.gpsimd.tensor_mul(n1a[:, 0, :], x_sb[:, 2, :], x_sb[:, 2, :])
            nc.gpsimd.tensor_mul(n1a[:, 1, :], x_sb[:, 3, :], x_sb[:, 3, :])
            n1b = tmp.tile([P, ST], FP32, tag="n1b")
            nc.gpsimd.tensor_add(n1b[:], sq_sb[:, 1, :], n1a[:, 0, :])
            n1 = tmp.tile([P, ST], FP32, tag="n1", bufs=2)
            nc.gpsimd.tensor_add(n1[:], n1b[:], n1a[:, 1, :])

            # xa = sum_m a1m * x1m (independent of c1, so the late
            # s = c1*xa becomes a single op instead of a 5-op tail)
            xam = tmp.tile([P, 3, ST], FP32, tag="sm")
            nc.gpsimd.tensor_mul(xam[:, 0, :], x_sb[:, 1, :], a_sb[:, 1, :])
            nc.gpsimd.tensor_mul(xam[:, 1, :], x_sb[:, 2, :], a_sb[:, 2, :])
            nc.gpsimd.tensor_mul(xam[:, 2, :], x_sb[:, 3, :], a_sb[:, 3, :])
            xa2 = tmp.tile([P, ST], FP32, tag="sa")
            nc.gpsimd.tensor_add(xa2[:], xam[:, 0, :], xam[:, 1, :])
            xa = tmp.tile([P, ST], FP32, tag="xa", bufs=2)
            nc.gpsimd.tensor_add(xa[:], xa2[:], xam[:, 2, :])

            # ---------------- y0 via Horner: y0 = x0*A + n1*B ----------------
            # A = wz0_0 + wz0_1*x0 + wz0_3*x0sq ; B = wz0_2 + wz0_4*x0
            Ap = accA.tile([P, ST], FP32, tag="accA")
            nc.tensor.matmul(Ap[:], wsc_sb[:, 0:128], attrs_T, start=True, stop=False)
            Bp = accA.tile([P, ST], FP32, tag="accA")
            nc.tensor.matmul(Bp[:], wsc_sb[:, 256:384], attrs_T, start=True, stop=False)
            wz1t = wzp.tile([P, ST], FP32, tag="wz")
            nc.tensor.matmul(wz1t[:], wsc_sb[:, 128:256], attrs_T)
            u1 = tmp.tile([P, ST], F32R, tag="u1", bufs=2)
            nc.vector.tensor_mul(u1[:], wz1t[:], x0)
            wz3t = wzp.tile([P, ST], FP32, tag="wz")
            nc.tensor.matmul(wz3t[:], wsc_sb[:, 384:512], attrs_T)
            u2 = tmp.tile([P, ST], F32R, tag="u2", bufs=2)
            nc.vector.tensor_mul(u2[:], wz3t[:], x0sq)
            wz4t = wzp.tile([P, ST], FP32, tag="wz")
            nc.tensor.matmul(wz4t[:], wsc_sb[:, 512:640], attrs_T)
            u3 = tmp.tile([P, ST], F32R, tag="u3")
            nc.vector.tensor_mul(u3[:], wz4t[:], x0)
            nc.tensor.matmul(Ap[:], ident[:], u1[:], start=False, stop=False)
            nc.tensor.matmul(Ap[:], ident[:], u2[:], start=False, stop=True)
            nc.tensor.matmul(Bp[:], ident[:], u3[:], start=False, stop=True)
            xA = tmp.tile([P, ST], F32R, tag="xA")
            nc.vector.tensor_mul(xA[:], Ap[:], x0)
            nB = tmp.tile([P, ST], F32R, tag="nB")
            nc.vector.tensor_mul(nB[:], Bp[:], n1[:])
            y0p = accA.tile([P, ST], FP32, tag="accA")
            nc.tensor.matmul(y0p[:], ident[:], xA[:], start=True, stop=False)
            nc.tensor.matmul(y0p[:], ident[:], nB[:], start=False, stop=True)
            y0 = tmp.tile([P, ST], F32R, tag="y0", bufs=2)
            nc.scalar.copy(y0[:], y0p[:])

            # ---------------- c1 = wz1_0 + wz1_1 x0 + wz1_2 x0^2 + wz1_3 n1 -
            c1p = accA.tile([P, ST], FP32, tag="accA")
            nc.tensor.matmul(
                c1p[:], wsc_sb[:, 640:768], attrs_T, start=True, stop=False
            )
            cmon = [x0, x0sq, n1[:]]
            ms = []
            for i_ in range(3):
                wz = wzp.tile([P, ST], FP32, tag="wz")
                nc.tensor.matmul(
                    wz[:], wsc_sb[:, 768 + i_ * P : 768 + (i_ + 1) * P], attrs_T
                )
                mt = tmp.tile([P, ST], F32R, tag=f"m{i_}", bufs=2)
                nc.vector.tensor_mul(mt[:], wz[:], cmon[i_])
                ms.append(mt)
            for i_ in range(3):
                nc.tensor.matmul(
                    c1p[:], ident[:], ms[i_][:],
                    start=False, stop=(i_ == 2),
                )
            c1 = tmp.tile([P, ST], FP32, tag="c1")
            nc.scalar.copy(c1[:], c1p[:])

            # ---------------- y1m = c1 * x1m ----------------
            y1 = tmp.tile([P, 3, ST], F32R, tag="y1", bufs=2)
            nc.vector.tensor_mul(y1[:], bcast_mid(c1p[:], 3), x1)

            # ---------------- s = c1 * xa ----------------
            s_ = tmp.tile([P, ST], FP32, tag="s_")
            nc.vector.tensor_mul(s_[:], c1p[:], xa[:])

            # tpw blocks wa/wb/wc/wd stay in PSUM; DVE consumers read them there
            tpw_ps = []
            for k in range(4):
                tp = wzp.tile([P, ST], FP32, tag="wz")
                nc.tensor.matmul(
                    tp[:], w234_sb[:, 128 + k * P : 128 + (k + 1) * P], h3[:]
                )
                tpw_ps.append(tp)
            wap, wbp, wcp, wdp = tpw_ps

            # ---------------- TP products ----------------
            wc = tmp.tile([P, ST], FP32, tag="wc_sb", bufs=2)
            nc.scalar.copy(wc[:], wcp[:])
            ay0 = tmp.tile([P, ST], FP32, tag="ay0")
            nc.gpsimd.tensor_mul(ay0[:], a0, y0[:])
            g1a = tmp.tile([P, ST], F32R, tag="g1a", bufs=2)
            nc.vector.tensor_mul(g1a[:], ay0[:], wap[:])
            g2 = tmp.tile([P, ST], F32R, tag="g2", bufs=2)
            nc.vector.tensor_mul(g2[:], wbp[:], s_[:])
            pc2 = tmp.tile([P, ST], FP32, tag="pc2")
            nc.gpsimd.tensor_mul(pc2[:], wc[:], y0[:])
            m1c = tmp.tile([P, 3, ST], F32R, tag="m1c")
            for m in range(3):
                nc.gpsimd.tensor_mul(m1c[:, m, :], pc2[:], a_sb[:, 1 + m, :])
            rc = tmp.tile([P, ST], FP32, tag="rc")
            nc.vector.tensor_mul(rc[:], wdp[:], a0)
            rc2 = tmp.tile([P, ST], FP32, tag="rc2")
            nc.gpsimd.tensor_mul(rc2[:], rc[:], c1[:])
            hm = tmp.tile([P, 3, ST], F32R, tag="hm")
            for m in range(3):
                nc.gpsimd.tensor_mul(hm[:, m, :], rc2[:], x_sb[:, 1 + m, :])

            # ---------------- output linears (feature-major accumulation) ---
            outT_sb = stream.tile([P, 4, ST], FP32, tag="outT")
            o0 = op_ps.tile([P, ST], FP32, tag="op")
            nc.tensor.matmul(o0[:], WA0, g1a[:], start=True, stop=False)
            nc.tensor.matmul(o0[:], WB0, g2[:], start=False, stop=False)
            nc.tensor.matmul(o0[:], WO0, y0[:], start=False, stop=True)
            oc0 = tmp.tile([P, ST], FP32, tag="oc0")
            nc.scalar.copy(oc0[:], o0[:])
            nc.gpsimd.tensor_add(outT_sb[:, 0, :], oc0[:], sc_sb[:, 0, :])
            for m in range(3):
                om = op_ps.tile([P, ST], FP32, tag="op")
                nc.tensor.matmul(om[:], WA1, m1c[:, m, :], start=True, stop=False)
                nc.tensor.matmul(om[:], WB1, hm[:, m, :], start=False, stop=False)
                nc.tensor.matmul(om[:], WO1, y1[:, m, :], start=False, stop=True)
                oc = tmp.tile([P, ST], FP32, tag=f"oc{m}")
                nc.scalar.copy(oc[:], om[:])
                nc.gpsimd.tensor_add(outT_sb[:, 1 + m, :], oc[:], sc_sb[:, 1 + m, :])

            pending_out = (outT_sb, lo)

        po_sb, po_lo = pending_out
        nc.sync.dma_start(out=out_r[:, :, po_lo : po_lo + ST], in_=po_sb[:])

    nc.compile()
    return nc


_CACHE = {}


def _get_program(n_st):
    if n_st not in _CACHE:
        _CACHE[n_st] = build_program(n_st)
    return _CACHE[n_st]


def _in_map_for_core(inputs, c, n_core):
    lo, hi = c * n_core, (c + 1) * n_core
    nf = np.asarray(inputs["node_feats"][lo:hi], dtype=np.float32)  # [n, C, 4]
    sc = np.asarray(inputs["sc"][lo:hi], dtype=np.float32)  # [n, 4C]
    nfT = np.ascontiguousarray(nf.transpose(2, 1, 0))  # [4, C, n]
    sc0 = sc[:, :C].T[None]  # [1, C, n]
    sc1 = np.ascontiguousarray(sc[:, C:].reshape(-1, C, 3).transpose(2, 1, 0))
    scT = np.ascontiguousarray(np.concatenate([sc0, sc1], axis=0))  # [4, C, n]
    n_loc = hi - lo
    smallT = np.zeros((68, n_loc), dtype=np.float32)
    smallT[0:E] = np.asarray(inputs["node_attrs"][lo:hi], dtype=np.float32).T
    smallT[32 : 32 + INV] = np.asarray(
        inputs["magmom_node_inv_feats"][lo:hi], dtype=np.float32
    ).T
    smallT[64:68] = np.asarray(inputs["magmom_node_attrs"][lo:hi], dtype=np.float32).T
    wsc = np.ascontiguousarray(
        np.concatenate(
            [
                np.asarray(inputs["w_sc0"], dtype=np.float32).reshape(E, 5 * C),
                np.asarray(inputs["w_sc1"], dtype=np.float32).reshape(E, 4 * C),
            ],
            axis=1,
        )
    )  # [10, 1152]
    w234 = np.ascontiguousarray(
        np.concatenate(
            [
                np.asarray(inputs["w_mlp2"], dtype=np.float32),
                np.asarray(inputs["w_mlp3"], dtype=np.float32),
                np.asarray(inputs["w_mlp4"], dtype=np.float32),
            ],
            axis=1,
        )
    )  # [64, 640]
    wl0 = np.asarray(inputs["W_l0"], dtype=np.float32)
    wl1 = np.asarray(inputs["W_l1"], dtype=np.float32)
    wf = np.ascontiguousarray(
        np.concatenate(
            [
                wl0[:C],
                wl0[C:],
                np.asarray(inputs["Wo0"], dtype=np.float32),
                wl1[:C],
                wl1[C:],
                np.asarray(inputs["Wo1"], dtype=np.float32),
            ],
            axis=1,
        )
    )  # [128, 768]
    return {
        "nfT": nfT,
        "scT": scT,
        "smallT": smallT,
        "wsc": wsc,
        "w1": np.ascontiguousarray(np.asarray(inputs["w_mlp1"], dtype=np.float32)),
        "w234": w234,
        "wf": wf,
    }


def _out_from_core(outT):
    """outT [4, C, n] -> out [n, 4C] in reference layout."""
    n = outT.shape[2]
    out = np.empty((n, 4 * C), dtype=np.float32)
    out[:, :C] = outT[0].T
    out[:, C:] = outT[1:4].transpose(2, 1, 0).reshape(n, 3 * C)
    return out


def run_on_hw(inputs, trace=False):
    n_nodes = np.asarray(inputs["node_feats"]).shape[0]
    n_core = n_nodes // N_CORES
    nc = _get_program(n_core // ST)
    in_maps = [_in_map_for_core(inputs, c, n_core) for c in range(N_CORES)]
    res = run_bass_kernel_spmd(
        nc, in_maps, core_ids=list(range(N_CORES)), trace=trace
    )
    out = np.concatenate(
        [_out_from_core(res.results[c]["outT"]) for c in range(N_CORES)], axis=0
    )
    return out.astype(np.float32), res


def kernel(**inputs) -> np.ndarray:
    import os, time

    os.environ.setdefault("NEURON_RT_RESET_CORES", "1")
    last = None
    for attempt in range(3):
        try:
            out, _ = run_on_hw(inputs, trace=False)
            return out
        except Exception as e:
            last = e
            time.sleep(5 + 5 * attempt)
    raise last


def bench(inputs, iters=5):
    """Pipelined timing of the sharded NEFF execution (device-resident inputs)."""
    import time
    import jax
    from jax.sharding import Mesh, PartitionSpec
    from jax.experimental.shard_map import shard_map
    from concourse import bass2jax
    from concourse.bass2jax import _bass_exec_p, install_neuronx_cc_hook

    inputs = {k: np.asarray(v, dtype=np.float32) for k, v in inputs.items()}
    n_nodes = inputs["node_feats"].shape[0]
    n_core = n_nodes // N_CORES
    nc = _get_program(n_core // ST)
    in_maps = [_in_map_for_core(inputs, c, n_core) for c in range(N_CORES)]

    install_neuronx_cc_hook()
    partition_name = nc.partition_id_tensor.name if nc.partition_id_tensor else None
    in_names, out_names, out_avals, zero_outs = [], [], [], []
    for alloc in nc.m.functions[0].allocations:
        if not isinstance(alloc, mybir.MemoryLocationSet):
            continue
        name = alloc.memorylocations[0].name
        if alloc.kind == "ExternalInput":
            if name != partition_name:
                in_names.append(name)
        elif alloc.kind == "ExternalOutput":
            out_names.append(name)
            shape = tuple(alloc.tensor_shape)
            dtype = mybir.dt.np(alloc.dtype)
            out_avals.append(jax.core.ShapedArray(shape, dtype))
            zero_outs.append(np.zeros(shape, dtype))
    n_params = len(in_names)
    all_names = in_names + out_names
    if partition_name is not None:
        all_names.append(partition_name)

    def _body(*args):
        operands = list(args)
        if partition_name is not None:
            operands.append(bass2jax.partition_id_tensor())
        return tuple(
            _bass_exec_p.bind(
                *operands,
                out_avals=tuple(out_avals),
                in_names=tuple(all_names),
                out_names=tuple(out_names),
                lowering_input_output_aliases=(),
                sim_require_finite=True,
                sim_require_nnan=True,
                nc=nc,
            )
        )

    devices = jax.devices()[:N_CORES]
    mesh = Mesh(np.asarray(devices), ("core",))
    nin = n_params + len(out_names)
    sharded = jax.jit(
        shard_map(
            _body,
            mesh=mesh,
            in_specs=(PartitionSpec("core"),) * nin,
            out_specs=(PartitionSpec("core"),) * len(out_names),
            check_rep=False,
        ),
        keep_unused=True,
    )
    per_core = [[np.asarray(m[nm]) for nm in in_names] for m in in_maps]
    concat_in = [
        np.concatenate([per_core[c][i] for c in range(N_CORES)], axis=0)
        for i in range(n_params)
    ]
    concat_zeros = [
        np.zeros((N_CORES * z.shape[0], *z.shape[1:]), z.dtype) for z in zero_outs
    ]
    from jax.sharding import NamedSharding
    sh = NamedSharding(mesh, PartitionSpec("core"))
    dev_in = [jax.device_put(a, sh) for a in concat_in + concat_zeros]
    out = sharded(*dev_in)
    jax.block_until_ready(out)
    t0 = time.time()
    for _ in range(iters):
        out = sharded(*dev_in)
    jax.block_until_ready(out)
    dt = (time.time() - t0) / iters
    return dt * 1e9, out



def bench_chain(inputs, n_chain=16, loops=3):
    """Per-execution time with N NEFF executions chained inside ONE jitted
    program: amortizes host/tunnel dispatch so total/N approaches true device
    execution time. Returns best ns/exec over `loops` timed dispatches."""
    import time
    import jax
    from jax.sharding import Mesh, PartitionSpec, NamedSharding
    from jax.experimental.shard_map import shard_map
    from concourse import bass2jax
    from concourse.bass2jax import _bass_exec_p, install_neuronx_cc_hook

    inputs = {k: np.asarray(v, dtype=np.float32) for k, v in inputs.items()}
    n_nodes = np.asarray(inputs["node_feats"]).shape[0]
    n_core = n_nodes // N_CORES
    nc = _get_program(n_core // ST)
    in_maps = [_in_map_for_core(inputs, c, n_core) for c in range(N_CORES)]

    install_neuronx_cc_hook()
    partition_name = nc.partition_id_tensor.name if nc.partition_id_tensor else None
    in_names, out_names, out_avals, zero_outs = [], [], [], []
    for alloc in nc.m.functions[0].allocations:
        if not isinstance(alloc, mybir.MemoryLocationSet):
            continue
        name = alloc.memorylocations[0].name
        if alloc.kind == "ExternalInput":
            if name != partition_name:
                in_names.append(name)
        elif alloc.kind == "ExternalOutput":
            out_names.append(name)
            shape = tuple(alloc.tensor_shape)
            dtype = mybir.dt.np(alloc.dtype)
            out_avals.append(jax.core.ShapedArray(shape, dtype))
            zero_outs.append(np.zeros(shape, dtype))
    n_params = len(in_names)
    all_names = in_names + out_names
    if partition_name is not None:
        all_names.append(partition_name)

    def _one(args):
        operands = list(args)
        if partition_name is not None:
            operands.append(bass2jax.partition_id_tensor())
        return _bass_exec_p.bind(
            *operands,
            out_avals=tuple(out_avals),
            in_names=tuple(all_names),
            out_names=tuple(out_names),
            lowering_input_output_aliases=(),
            sim_require_finite=True,
            sim_require_nnan=True,
            nc=nc,
        )

    def _body(*args):
        outs = None
        for _ in range(n_chain):
            outs = _one(args)
        return tuple(outs)

    devices = jax.devices()[:N_CORES]
    mesh = Mesh(np.asarray(devices), ("core",))
    nin = n_params + len(out_names)
    sharded = jax.jit(
        shard_map(
            _body,
            mesh=mesh,
            in_specs=(PartitionSpec("core"),) * nin,
            out_specs=(PartitionSpec("core"),) * len(out_names),
            check_rep=False,
        ),
        keep_unused=True,
    )
    per_core = [[np.asarray(m[nm]) for nm in in_names] for m in in_maps]
    concat_in = [
        np.concatenate([per_core[c][i] for c in range(N_CORES)], axis=0)
        for i in range(n_params)
    ]
    concat_zeros = [
        np.zeros((N_CORES * z.shape[0], *z.shape[1:]), z.dtype) for z in zero_outs
    ]
    sh = NamedSharding(mesh, PartitionSpec("core"))
    dev_in = [jax.device_put(a, sh) for a in concat_in + concat_zeros]
    out = sharded(*dev_in)
    jax.block_until_ready(out)
    best = None
    for _ in range(loops):
        t0 = time.time()
        out = sharded(*dev_in)
        jax.block_until_ready(out)
        dt = (time.time() - t0) / n_chain
        if best is None or dt < best:
            best = dt
    return best * 1e9, out
